# revision 1
# baseline (speedup 1.0000x reference)
"""Trainium2 Bass kernel for MultiHeadLatentAttention (B=2, S=2048, H=2048,
NH=16, HD=128, LAT=512), SPMD across 8 NeuronCores.

Sharding: 8 cores = 2 (batch) x 4 (head-group TP). Core c handles batch c//4
and head group j = c%4 = heads {j, 4+j, 8+j, 12+j}. That grouping is chosen so
the 4 heads share exactly 256 rows of Wq_up/Wk_up: heads j and 4+j are the raw
x1/x2 slices of q_half, heads 8+j and 12+j are their RoPE combinations — so
the up-projection shards 4-way with no duplication. Each core computes its
partial o_proj output; the host sums the 4 partials per batch and adds bo.

Self-contained: builds + compiles the Bass program on first call (cached),
runs via run_bass_kernel_spmd on cores 0-7.
"""
import os
import sys
import types
from contextlib import ExitStack

import numpy as np

if "/opt/trn_rl_repo" not in sys.path:
    sys.path.insert(0, "/opt/trn_rl_repo")

import ml_dtypes

# ---------------------------------------------------------------------------
# NTFF-profile shim: antenv.axon_hooks is missing in this image; register a
# hook backed by the axon PJRT .so so trace=True can capture HW exec time.
# ---------------------------------------------------------------------------


def _install_axon_hooks_shim():
    if "antenv.axon_hooks" in sys.modules:
        return
    try:
        import antenv
        from trn_agent_boot.trn_boot import _ntff_profile_via_ctypes
        hook = _ntff_profile_via_ctypes("/opt/axon/libaxon_pjrt.so")
    except Exception:
        return
    mod = types.ModuleType("antenv.axon_hooks")
    mod.get_axon_ntff_profile_hook = lambda: hook
    mod.set_axon_ntff_profile_hook = lambda h: None
    sys.modules["antenv.axon_hooks"] = mod
    antenv.axon_hooks = mod


_install_axon_hooks_shim()

import concourse.bass as bass  # noqa: E402
import concourse.mybir as mybir  # noqa: E402
import concourse.tile as tile  # noqa: E402
from concourse import bacc  # noqa: E402
from concourse.bass_utils import run_bass_kernel_spmd  # noqa: E402

P = 128
H = 2048
NH = 16
HD = 128
LAT = 512
B = 2
S = 2048
ROPE_DIM = H // 4
NHG = 4          # heads per core
SC = 512         # s/q chunk (one PSUM bank of fp32)
INV_SQRT_HD = 0.08838834764831845  # 1/sqrt(128)

f32 = mybir.dt.float32
f32r = mybir.dt.float32r
bf16 = mybir.dt.bfloat16
f16 = mybir.dt.float16
Act = mybir.ActivationFunctionType
Alu = mybir.AluOpType
BF16 = ml_dtypes.bfloat16
F16 = np.float16


def build_mla(seq=S, debug=False):
    """Build one core's program. All cores run this same program SPMD."""
    NSC = seq // SC   # s-chunks
    HT = H // P       # 16 h-tiles
    LT = LAT // P     # 4 l-tiles
    ST = seq // P     # s-tiles (= k-tiles in attention)

    nc = bacc.Bacc("TRN2", target_bir_lowering=False, debug=debug)

    hsT = nc.dram_tensor("hsT", [H, seq], f16, kind="ExternalInput")
    WqeT = nc.dram_tensor("WqeT", [H, 2 * P], f16, kind="ExternalInput")
    WkvdT = nc.dram_tensor("WkvdT", [H, LAT], f16, kind="ExternalInput")
    bkvd = nc.dram_tensor("bkvd", [LAT], f32, kind="ExternalInput")
    WkuT = nc.dram_tensor("WkuT", [LAT, 2 * P], f32r, kind="ExternalInput")
    bqku = nc.dram_tensor("bqku", [P, 4], f32, kind="ExternalInput")
    WvuT = nc.dram_tensor("WvuT", [LAT, NHG * P], f32r, kind="ExternalInput")
    bvu = nc.dram_tensor("bvu", [1, NHG * P], f32, kind="ExternalInput")
    WoT = nc.dram_tensor("WoT", [NHG * P, H], f16, kind="ExternalInput")
    cosT = nc.dram_tensor("cosT", [P, seq], f16, kind="ExternalInput")
    sinT = nc.dram_tensor("sinT", [P, seq], f16, kind="ExternalInput")
    outT = nc.dram_tensor("outT", [H, seq], f16, kind="ExternalOutput")

    def r(ap):  # fast fp32 matmul path
        return ap.bitcast(f32r)

    with tile.TileContext(nc) as tc, ExitStack() as top:
        const = top.enter_context(tc.tile_pool(name="const", bufs=1))
        ao_pool = top.enter_context(tc.tile_pool(name="ao", bufs=1))

        bkvd_t = const.tile([P, LT], f32)
        nc.sync.dma_start(bkvd_t[:], bkvd.rearrange("(o p) -> p o", p=P))
        # on-chip ones: no DMA dependency, so the HAM warmup starts as soon
        # as the DVE is live
        ones_r = const.tile([P, P], f16)
        nc.vector.memset(ones_r[:], 1.0)

        # HAM warmup: ~64 back-to-back matmuls (~3.5us of PE activity) while
        # the initial weight/activation DMAs stream in, so the first real
        # matmuls run at 2.4GHz instead of the cold 1.2GHz.
        with tc.tile_pool(name="warm", bufs=1, space="PSUM") as warm_pool:
            wtiles = [warm_pool.tile([P, P], f32, tag=f"w{i}", name=f"warm{i}")
                      for i in range(4)]
            for i in range(144):
                nc.tensor.matmul(wtiles[i % 4][:], ones_r[:], ones_r[:],
                                 start=True, stop=True)

        attn_outT = ao_pool.tile([P, NHG, seq], f16)

        with ExitStack() as qkv_scope:
            qk_pool = qkv_scope.enter_context(tc.tile_pool(name="qk", bufs=1))
            v_pool = qkv_scope.enter_context(tc.tile_pool(name="v", bufs=1))
            qT = qk_pool.tile([P, NHG, seq], f16)  # 0=x1, 1=x2, 2,3=rope
            kT = qk_pool.tile([P, NHG, seq], f16)
            v_bf = v_pool.tile([P, ST, NHG * P], f16)  # token-major v

            with ExitStack() as lat_scope:
                lat_pool = lat_scope.enter_context(
                    tc.tile_pool(name="lat", bufs=1))
                kv_latT = lat_pool.tile([P, LT, seq], f32r)
                # early-U pool + U psum hoisted ABOVE the D pools: v-up can
                # start the moment the last D chain drains, instead of
                # waiting for D's pools to release and ~2.5MB of U DMAs.
                ue_pool = lat_scope.enter_context(
                    tc.tile_pool(name="uearly", bufs=1))
                psu = lat_scope.enter_context(
                    tc.tile_pool(name="psu", bufs=4, space="PSUM"))
                bqku_t = ue_pool.tile([P, 4], f32)
                bvu_bc = ue_pool.tile([P, NHG * P], f32)
                wvu_t = ue_pool.tile([P, LT, NHG * P], f32r)
                cos_t = ue_pool.tile([P, seq], f16)
                sin_t = ue_pool.tile([P, seq], f16)
                wku_t = ue_pool.tile([P, LT, 2 * P], f32r)
                ut_pool = lat_scope.enter_context(
                    tc.tile_pool(name="ut", bufs=4))

                # ---------------- phase D: projections from hs -------------
                # per s-chunk: 4 kv_lat chains + 2 fused-q chains. The q
                # down+up pair is algebraically folded into one [H, 256]
                # effective weight (Wqd.T @ Wqu_sel.T) on the host, so the
                # duplicated q-down never runs on-device: 27us instead of
                # 61us of PE per core. q_half lands directly in qT.
                with tc.tile_pool(name="wd", bufs=1) as wd_pool, \
                     tc.tile_pool(name="hst", bufs=2 * HT + 16) as hst_pool, \
                     tc.tile_pool(name="psd", bufs=4, space="PSUM") as psd:
                    # load order: first s-chunk of activations, then
                    # Wkv_down (chains m=0..3 need it), then WqeT (m=4,5)
                    wkvd_t = wd_pool.tile([P, HT, LAT], f16)
                    wqe_t = wd_pool.tile([P, HT, 2 * P], f16)
                    hts0 = []
                    for ht in range(HT):
                        t = hst_pool.tile([P, SC], f16, tag="hst")
                        nc.sync.dma_start(t[:], hsT[ht * P:(ht + 1) * P, :SC])
                        hts0.append(t)
                    for ht in range(HT):
                        nc.sync.dma_start(
                            wkvd_t[:, ht, :], WkvdT[ht * P:(ht + 1) * P, :])
                    for ht in range(HT):
                        nc.sync.dma_start(
                            wqe_t[:, ht, :], WqeT[ht * P:(ht + 1) * P, :])
                    nc.sync.dma_start(bqku_t[:], bqku[:])
                    nc.sync.dma_start(
                        bvu_bc[:], bvu[:].to_broadcast((P, NHG * P)))
                    nc.sync.dma_start(
                        wvu_t[:], WvuT.rearrange("(lt p) m -> p lt m", p=P))
                    nc.sync.dma_start(
                        wku_t[:], WkuT.rearrange("(lt p) m -> p lt m", p=P))
                    nc.sync.dma_start(cos_t[:], cosT[:])
                    nc.sync.dma_start(sin_t[:], sinT[:])

                    for sc in range(NSC):
                        ssl = slice(sc * SC, (sc + 1) * SC)
                        if sc == 0:
                            hts = hts0
                        else:
                            hts = []
                            for ht in range(HT):
                                t = hst_pool.tile([P, SC], f16, tag="hst")
                                nc.sync.dma_start(
                                    t[:], hsT[ht * P:(ht + 1) * P, ssl])
                                hts.append(t)
                        for m in range(LT + 2):
                            ps = psd.tile([P, SC], f32)
                            if m < LT:
                                w_sl = wkvd_t[:, :, m * P:(m + 1) * P]
                            else:
                                ci = m - LT
                                w_sl = wqe_t[:, :, ci * P:(ci + 1) * P]
                            for ht in range(HT):
                                nc.tensor.matmul(
                                    ps[:], w_sl[:, ht, :], hts[ht][:],
                                    start=(ht == 0), stop=(ht == HT - 1))
                            if m < LT:
                                nc.scalar.activation(
                                    kv_latT[:, m, ssl], ps[:], Act.Identity,
                                    bias=bkvd_t[:, m:m + 1])
                            else:
                                # fused q: bias col ci of bqku = b_eff slice
                                nc.scalar.activation(
                                    qT[:, ci, ssl], ps[:], Act.Identity,
                                    bias=bqku_t[:, ci:ci + 1])

                        # phase U dissolved into the sc loop: this chunk's
                        # k-up, v-up and rope run right here, so there is
                        # no separate U phase or boundary stall, and the
                        # hst tiles for chunk sc+1 get a longer prefetch
                        # window.
                        for ci in (2, 3):  # k_x1, k_x2
                            csl = slice((ci % 2) * P, (ci % 2) * P + P)
                            ps = psu.tile([P, SC], f32, tag="psu")
                            for lt in range(LT):
                                nc.tensor.matmul(
                                    ps[:], wku_t[:, lt, csl],
                                    kv_latT[:, lt, ssl],
                                    start=(lt == 0), stop=(lt == LT - 1))
                            # bias-add on DVE (free-dim broadcast of [P,1])
                            # to keep ACT free for the attention exps
                            nc.vector.tensor_tensor(
                                kT[:, ci % 2, ssl], ps[:],
                                bqku_t[:, ci:ci + 1].to_broadcast((P, SC)),
                                Alu.add)

                        for stl in range(SC // P):
                            st = sc * (SC // P) + stl
                            ps = psu.tile([P, NHG * P], f32, tag="psu")
                            for lt in range(LT):
                                nc.tensor.matmul(
                                    ps[:],
                                    kv_latT[:, lt, st * P:(st + 1) * P],
                                    wvu_t[:, lt, :],
                                    start=(lt == 0), stop=(lt == LT - 1))
                            nc.vector.tensor_tensor(
                                v_bf[:, st, :], ps[:], bvu_bc[:], Alu.add)

                        # rope: slot2 = x1*cos - x2*sin,
                        #       slot3 = x1*sin + x2*cos
                        for dstT in (kT, qT):
                            x1 = dstT[:, 0, ssl]
                            x2 = dstT[:, 1, ssl]
                            t1 = ut_pool.tile([P, SC], f16, tag="ropetmp")
                            t2 = ut_pool.tile([P, SC], f16, tag="ropetmp")
                            nc.vector.tensor_mul(t1[:], x1, cos_t[:, ssl])
                            nc.vector.tensor_mul(t2[:], x2, sin_t[:, ssl])
                            nc.vector.tensor_sub(dstT[:, 2, ssl], t1[:],
                                                 t2[:])
                            t3 = ut_pool.tile([P, SC], f16, tag="ropetmp")
                            t4 = ut_pool.tile([P, SC], f16, tag="ropetmp")
                            nc.vector.tensor_mul(t3[:], x1, sin_t[:, ssl])
                            nc.vector.tensor_mul(t4[:], x2, cos_t[:, ssl])
                            nc.vector.tensor_add(dstT[:, 3, ssl], t3[:],
                                                 t4[:])

            # ------- phase A+O: attention with o_proj interleaved --------
            # qc-outer: once all 4 heads of a q-chunk are normalized, that
            # chunk's o_proj runs on PE underneath the next chunk's
            # ACT-bound score/exp pipeline.
            with tc.tile_pool(name="exp", bufs=2) as exp_pool, \
                 tc.tile_pool(name="wo", bufs=1) as wo_pool, \
                 tc.tile_pool(name="pss", bufs=2, space="PSUM") as pss, \
                 tc.tile_pool(name="psav", bufs=1, space="PSUM") as psav, \
                 tc.tile_pool(name="pssm", bufs=1, space="PSUM") as pssm, \
                 tc.tile_pool(name="pso", bufs=2, space="PSUM") as pso, \
                 tc.tile_pool(name="att", bufs=3) as at_pool, \
                 tc.tile_pool(name="ot", bufs=4) as ot_pool:
                # o_proj weights resident: [c-part, ct, m] layout
                wo_t = wo_pool.tile([P, NHG, H], f16)
                for ct in range(NHG):
                    nc.sync.dma_start(
                        wo_t[:, ct, :], WoT[ct * P:(ct + 1) * P, :])

                def scores_exp(qc, h):
                    qsl = slice(qc * SC, (qc + 1) * SC)
                    expt = exp_pool.tile([P, ST, SC], f16, tag="expt",
                                         name="expt")
                    # k-tiles in pairs: two matmuls fill a 2-bank psum
                    # tile, one ACT exp covers both (amortizes the ~240ns
                    # per-ACT-instruction overhead). The softmax
                    # denominator accumulates incrementally right behind
                    # each exp group, so it is ready ~one group after the
                    # last exp instead of 15 serial adds later.
                    acc = at_pool.tile([P, SC], f16, tag="acc", name="acc")
                    for kth in range(ST // 2):
                        ps = pss.tile([P, 2, SC], f32, tag="score",
                                      name="score")
                        for half in (0, 1):
                            kt = 2 * kth + half
                            nc.tensor.matmul(
                                ps[:, half, :],
                                kT[:, h, kt * P:(kt + 1) * P],
                                qT[:, h, qsl], start=True, stop=True)
                        nc.scalar.activation(
                            expt[:, 2 * kth:2 * kth + 2, :], ps[:],
                            Act.Exp, scale=INV_SQRT_HD)
                        if kth == 0:
                            nc.vector.tensor_add(
                                acc[:], expt[:, 0, :], expt[:, 1, :])
                        else:
                            nc.vector.tensor_add(
                                acc[:], acc[:], expt[:, 2 * kth, :])
                            nc.vector.tensor_add(
                                acc[:], acc[:], expt[:, 2 * kth + 1, :])
                    return expt, acc

                def av_norm(qc, h, expt, acc):
                    qsl = slice(qc * SC, (qc + 1) * SC)
                    # AV accumulation on PE
                    pav = psav.tile([P, SC], f32, tag="av", name="av")
                    for kt in range(ST):
                        nc.tensor.matmul(
                            pav[:], v_bf[:, kt, h * P:(h + 1) * P],
                            expt[:, kt, :],
                            start=(kt == 0), stop=(kt == ST - 1))
                    # one ones-matmul = 128-way partition reduce of the
                    # pre-accumulated denominators, broadcast to all
                    # partitions.
                    psm = pssm.tile([P, SC], f32, tag="sum", name="sum")
                    nc.tensor.matmul(
                        psm[:], ones_r[:], acc[:], start=True, stop=True)
                    rec_bc = at_pool.tile([P, SC], f32, tag="rec_bc",
                                          name="rec")
                    nc.vector.reciprocal_approx_fast(rec_bc[:], psm[:])
                    nc.vector.tensor_tensor(
                        attn_outT[:, h, qsl], pav[:], rec_bc[:], Alu.mult)

                def o_proj(qc):
                    qsl = slice(qc * SC, (qc + 1) * SC)
                    for mt in range(H // P):
                        msl = slice(mt * P, (mt + 1) * P)
                        ps = pso.tile([P, SC], f32, tag="opsum",
                                      name="opsum")
                        for ct in range(NHG):
                            nc.tensor.matmul(
                                ps[:], wo_t[:, ct, msl],
                                attn_outT[:, ct, qsl],
                                start=(ct == 0), stop=(ct == NHG - 1))
                        ot = ot_pool.tile([P, SC], f16, tag="ot", name="ot")
                        nc.vector.tensor_copy(ot[:], ps[:])
                        nc.sync.dma_start(outT[msl, qsl], ot[:])

                # software pipeline: AV(h) runs on PE behind scores(h+1),
                # so PE never waits on the exp tail of its own head; the
                # previous chunk's o_proj slides under the next chunk's
                # first score/exp wave.
                prev = None
                for qc in range(NSC):
                    for h in range(NHG):
                        expt, acc = scores_exp(qc, h)
                        if prev is not None:
                            av_norm(*prev)
                            if prev[1] == NHG - 1:
                                o_proj(prev[0])
                        prev = (qc, h, expt, acc)
                av_norm(*prev)
                o_proj(NSC - 1)

    nc.compile()
    return nc


# ---------------------------------------------------------------------------
# Host side: shard inputs, run SPMD, gather.
# ---------------------------------------------------------------------------

def _rope_cos_sin(seq_len, dim, base=10000.0):
    inv_freq = 1.0 / (base ** (np.arange(0, dim, 2, dtype=np.float32) / dim))
    t = np.arange(seq_len, dtype=np.float32)
    freqs = np.outer(t, inv_freq).astype(np.float32)
    emb = np.concatenate([freqs, freqs], -1)
    return np.cos(emb).astype(np.float32), np.sin(emb).astype(np.float32)


def make_in_maps(hidden_states, Wq_down, bq_down, Wkv_down, bkv_down,
                 Wq_up, bq_up, Wk_up, bk_up, Wv_up, bv_up, Wo, bo):
    cos, sin = _rope_cos_sin(S, ROPE_DIM)
    WkvdT = np.ascontiguousarray(Wkv_down.T).astype(F16)
    hsT = [np.ascontiguousarray(hidden_states[b].T).astype(F16)
           for b in range(B)]
    in_maps = []
    for c in range(8):
        b, j = c // 4, c % 4
        heads = [j, 4 + j, 8 + j, 12 + j]
        x1 = slice(j * P, (j + 1) * P)
        x2 = slice(512 + j * P, 512 + (j + 1) * P)
        vrows = np.concatenate(
            [np.arange(h * P, (h + 1) * P) for h in heads])
        # fused q path: q_half_sel = hs @ (Wqd.T @ Wqu_sel.T) + b_eff
        Wqu_sel = np.concatenate([Wq_up[x1], Wq_up[x2]], 0)  # [256, LAT]
        Wqe = (Wqu_sel.astype(np.float64)
               @ Wq_down.astype(np.float64))                 # [256, H]
        bqe = (Wqu_sel.astype(np.float64) @ bq_down.astype(np.float64)
               + np.concatenate([bq_up[x1], bq_up[x2]]).astype(np.float64))
        in_maps.append(dict(
            hsT=hsT[b],
            WkvdT=WkvdT,
            bkvd=np.ascontiguousarray(bkv_down),
            WqeT=np.ascontiguousarray(Wqe.T).astype(F16),
            WkuT=np.ascontiguousarray(
                np.concatenate([Wk_up[x1], Wk_up[x2]], 0).T),
            bqku=np.stack(
                [bqe[:P].astype(np.float32), bqe[P:].astype(np.float32),
                 bk_up[x1], bk_up[x2]], axis=1).copy(),
            WvuT=np.ascontiguousarray(Wv_up[vrows].T),
            bvu=np.ascontiguousarray(bv_up[vrows][None, :]),
            WoT=np.ascontiguousarray(Wo[:, vrows].T).astype(F16),
            cosT=np.ascontiguousarray(cos[:, x1].T).astype(F16),
            sinT=np.ascontiguousarray(sin[:, x1].T).astype(F16),
        ))
    return in_maps


_NC_CACHE = {}


def _get_nc():
    if "nc" not in _NC_CACHE:
        _NC_CACHE["nc"] = build_mla()
    return _NC_CACHE["nc"]


LAST_RESULTS = None  # BassKernelResults of the most recent kernel() call


def kernel(**inputs):
    global LAST_RESULTS
    nc = _get_nc()
    in_maps = make_in_maps(**inputs)
    trace = bool(int(os.environ.get("MLA_TRACE", "0")))
    kwargs = {}
    if trace:
        tc_env = os.environ.get("MLA_TRACE_CORES", "0,1,2,3,4,5,6,7")
        kwargs["trace_cores"] = [int(x) for x in tc_env.split(",")]
    res = run_bass_kernel_spmd(
        nc, in_maps, core_ids=list(range(8)), trace=trace, **kwargs)
    LAST_RESULTS = res
    bo = inputs["bo"]
    out = np.zeros((B, S, H), np.float32)
    for b in range(B):
        acc = res.results[b * 4]["outT"].astype(np.float32)
        for j in range(1, 4):
            acc = acc + res.results[b * 4 + j]["outT"]
        out[b] = acc.T + bo[None, :]
    return out

